# revision 3
# baseline (speedup 1.0000x reference)
"""Trainium2 Bass kernel for nn_LocalRelationalLayer_18262200943220.

The reference LocalRelationalLayer builds key/query maps and a softmax
composability tensor, but multiplies them into a feature map `fm` that is
identically zero (faithful to the torch original, see reference comment).
Everything upstream of the final 1x1x1 conv is therefore multiplied by
zero: out = einsum(zeros, f_w) + f_b == broadcast(f_b).

So the exact output is f_b broadcast to [1, 256, 14, 14, 128], bitwise
equal to the reference (verified). The kernel shards the 256 output
channels across the 8 NeuronCores (32 channels each); on-device each core
broadcast-fills its channel block and writes its 3.2 MB output slice.
"""

import numpy as np

import concourse.bass as bass
import concourse.mybir as mybir
from concourse.bass_utils import run_bass_kernel_spmd

OUT_SHAPE = (1, 256, 14, 14, 128)  # [B, outC, 2K, 2K, 2D] from the reference
OUTC = 256
SPATIAL = 14 * 14 * 128  # 25088 voxels per output channel
N_CORES = 8
CPC = OUTC // N_CORES  # 32 channels per core
P = 128  # SBUF partitions
REP = P // CPC  # 4 partitions per channel
COLS = SPATIAL // REP  # 6272 f32 per partition row
FILL_W = 1568  # SBUF fill width; the output DMA replicates it
N_REP_DMA = COLS // FILL_W  # 4 replications per partition row

_CACHE = {}


def _build_bass():
    """Per-core graph: out[p, :] = fb[p] for a [128, 6272] f32 block.

    Raw Bass (no Tile): two HWDGE DMAs on the sync sequencer plus one DVE
    broadcast fill, hand-synchronized. The output DMA reads the 1568-wide
    SBUF chunk four times per partition row (stride-0 middle dim).
    """
    f32 = mybir.dt.float32
    nc = bass.Bass("TRN2", debug=False)
    fb_in = nc.dram_tensor("fb", [P, 1], f32, kind="ExternalInput")
    out = nc.dram_tensor("out", [P, COLS], f32, kind="ExternalOutput")

    with (
        nc.sbuf_tensor("fb_sb", [P, 1], f32) as fb_sb,
        nc.sbuf_tensor("chunk", [P, FILL_W], f32) as chunk,
        nc.semaphore("dma_sem") as dma_sem,
        nc.semaphore("v_sem") as v_sem,
        nc.Block() as block,
    ):

        @block.sync
        def _(sync):
            sync.dma_start(out=fb_sb[:, :], in_=fb_in[:, :]).then_inc(dma_sem, 16)
            sync.wait_ge(v_sem, 1)
            out_v = out.ap().rearrange("p (r w) -> p r w", w=FILL_W)
            sync.dma_start(
                out=out_v, in_=chunk[:, None, :].broadcast_to([P, N_REP_DMA, FILL_W])
            ).then_inc(dma_sem, 16)
            sync.wait_ge(dma_sem, 32)

        @block.vector
        def _(vector):
            vector.wait_ge(dma_sem, 16)
            vector.tensor_copy(
                out=chunk[:, :], in_=fb_sb[:, 0:1].broadcast_to([P, FILL_W])
            ).then_inc(v_sem, 1)

    return nc


# Stashed BassKernelResults from the most recent run (exec_time_ns etc.);
# used by the dev harness, not by grading.
LAST_RUN = None


def kernel(**inputs) -> np.ndarray:
    global LAST_RUN
    f_b = np.ascontiguousarray(np.asarray(inputs["f_b"]), dtype=np.float32)
    assert f_b.shape == (OUTC,), f_b.shape

    # Shard channels across cores; lay each core's 32 channels out over all
    # 128 partitions (4 partitions per channel).
    in_maps = []
    for ci in range(N_CORES):
        shard = f_b[ci * CPC : (ci + 1) * CPC]
        in_maps.append({"fb": np.repeat(shard, REP).reshape(P, 1).copy()})

    if "nc" not in _CACHE:
        _CACHE["nc"] = _build_bass()
    res = run_bass_kernel_spmd(_CACHE["nc"], in_maps, core_ids=list(range(N_CORES)))
    LAST_RUN = res

    # Unshard: per-core [128, 6272] -> [32, 25088]; concat channel blocks.
    parts = [np.asarray(r["out"]).reshape(CPC, SPATIAL) for r in res.results]
    return np.concatenate(parts, axis=0).reshape(OUT_SHAPE)


# revision 5
# speedup vs baseline: 66430.3804x; 66430.3804x over previous
"""Trainium2 Bass kernel for nn_LocalRelationalLayer_18262200943220.

The reference LocalRelationalLayer builds key/query maps and a softmax
composability tensor, but multiplies them into a feature map `fm` that is
identically zero (faithful to the torch original, see reference comment).
Everything upstream of the final 1x1x1 conv is therefore multiplied by
zero: out = einsum(zeros, f_w) + f_b == broadcast(f_b).

So the exact output is f_b broadcast to [1, 256, 14, 14, 128], bitwise
equal to the reference (verified). The kernel shards the 256 output
channels across the 8 NeuronCores (32 channels each); on-device each core
broadcast-fills its channel block and writes its 3.2 MB output slice.
"""

import numpy as np

import concourse.bass as bass
import concourse.mybir as mybir
from concourse.bass_utils import run_bass_kernel_spmd

OUT_SHAPE = (1, 256, 14, 14, 128)  # [B, outC, 2K, 2K, 2D] from the reference
OUTC = 256
SPATIAL = 14 * 14 * 128  # 25088 voxels per output channel
N_CORES = 8
CPC = OUTC // N_CORES  # 32 channels per core
P = 128  # SBUF partitions
REP = P // CPC  # 4 partitions per channel
COLS = SPATIAL // REP  # 6272 f32 per partition row
FILL_W = 1568  # SBUF fill width; the output DMA replicates it
N_REP_DMA = COLS // FILL_W  # 4 replications per partition row

_CACHE = {}


FIRST_SLICE = 392  # first fill slice; its DMA starts while the rest fills


def _build_bass():
    """Per-core graph: out[p, :] = fb[p] for a [128, 6272] f32 block.

    Raw Bass (no Tile; Tile's kernel-tail Drain trips a codegen sync-wait
    limit here and its barriers cost more than this whole kernel).

    The SP sequencer issues the 512 B bias load; the DVE broadcast-fills a
    [128, 1568] SBUF chunk in two slices (392 + 1176 cols); SP issues the
    output DMAs as each slice lands: a small head DMA for cols [0:392), a
    mid DMA for [392:1568), and one bulk DMA for [1568:6272) that reads
    the chunk three times per partition row (stride-0 middle dim). Bulk
    descriptors are 6272 B, above the 4 KiB small-descriptor HBM penalty
    threshold. TimelineSim: ~15.3 us/core; HBM-write floor is ~11.1 us
    for the 3.2 MB slice, plus ~2.3 us input-DMA latency and ~1.3 us
    program start. Verified race-free and bit-exact in CoreSim.
    """
    f32 = mybir.dt.float32
    S0, W = FIRST_SLICE, FILL_W
    nc = bass.Bass("TRN2", debug=False)
    fb_in = nc.dram_tensor("fb", [P, 1], f32, kind="ExternalInput")
    out = nc.dram_tensor("out", [P, COLS], f32, kind="ExternalOutput")

    with (
        nc.sbuf_tensor("fb_sb", [P, 1], f32) as fb_sb,
        nc.sbuf_tensor("chunk", [P, W], f32) as chunk,
        nc.semaphore("dma_sem") as dma_sem,
        nc.semaphore("fill_sem") as fill_sem,
        nc.Block() as block,
    ):

        @block.sync
        def _(sync):
            sync.dma_start(out=fb_sb[:, :], in_=fb_in[:, :]).then_inc(dma_sem, 16)
            sync.wait_ge(fill_sem, 1)
            sync.dma_start(out=out.ap()[:, 0:S0], in_=chunk[:, 0:S0]).then_inc(
                dma_sem, 16
            )
            sync.wait_ge(fill_sem, 2)
            sync.dma_start(out=out.ap()[:, S0:W], in_=chunk[:, S0:W]).then_inc(
                dma_sem, 16
            )
            out_v = out.ap()[:, W:COLS].rearrange("p (r w) -> p r w", w=W)
            sync.dma_start(
                out=out_v,
                in_=chunk[:, None, :].broadcast_to([P, N_REP_DMA - 1, W]),
            ).then_inc(dma_sem, 16)
            sync.wait_ge(dma_sem, 64)

        @block.vector
        def _(vector):
            vector.wait_ge(dma_sem, 16)
            vector.tensor_copy(
                out=chunk[:, 0:S0], in_=fb_sb[:, 0:1].broadcast_to([P, S0])
            ).then_inc(fill_sem, 1)
            vector.tensor_copy(
                out=chunk[:, S0:W], in_=fb_sb[:, 0:1].broadcast_to([P, W - S0])
            ).then_inc(fill_sem, 1)

    return nc


# Stashed BassKernelResults from the most recent run (exec_time_ns etc.);
# used by the dev harness, not by grading.
LAST_RUN = None


def kernel(**inputs) -> np.ndarray:
    global LAST_RUN
    f_b = np.ascontiguousarray(np.asarray(inputs["f_b"]), dtype=np.float32)
    assert f_b.shape == (OUTC,), f_b.shape

    # Shard channels across cores; lay each core's 32 channels out over all
    # 128 partitions (4 partitions per channel).
    in_maps = []
    for ci in range(N_CORES):
        shard = f_b[ci * CPC : (ci + 1) * CPC]
        in_maps.append({"fb": np.repeat(shard, REP).reshape(P, 1).copy()})

    if "nc" not in _CACHE:
        _CACHE["nc"] = _build_bass()
    res = run_bass_kernel_spmd(_CACHE["nc"], in_maps, core_ids=list(range(N_CORES)))
    LAST_RUN = res

    # Unshard: per-core [128, 6272] -> [32, 25088]; concat channel blocks.
    parts = [np.asarray(r["out"]).reshape(CPC, SPATIAL) for r in res.results]
    return np.concatenate(parts, axis=0).reshape(OUT_SHAPE)


# revision 7
# speedup vs baseline: 66782.9463x; 1.0053x over previous
"""Trainium2 Bass kernel for nn_LocalRelationalLayer_18262200943220.

The reference LocalRelationalLayer builds key/query maps and a softmax
composability tensor, but multiplies them into a feature map `fm` that is
identically zero (faithful to the torch original, see reference comment).
Everything upstream of the final 1x1x1 conv is therefore multiplied by
zero: out = einsum(zeros, f_w) + f_b == broadcast(f_b).

So the exact output is f_b broadcast to [1, 256, 14, 14, 128], bitwise
equal to the reference (verified). The kernel shards the 256 output
channels across the 8 NeuronCores (32 channels each); on-device each core
broadcast-fills its channel block and writes its 3.2 MB output slice.
"""

import numpy as np

import concourse.bass as bass
import concourse.mybir as mybir
from concourse.bass_utils import run_bass_kernel_spmd

OUT_SHAPE = (1, 256, 14, 14, 128)  # [B, outC, 2K, 2K, 2D] from the reference
OUTC = 256
SPATIAL = 14 * 14 * 128  # 25088 voxels per output channel
N_CORES = 8
CPC = OUTC // N_CORES  # 32 channels per core
P = 128  # SBUF partitions
REP = P // CPC  # 4 partitions per channel
COLS = SPATIAL // REP  # 6272 f32 per partition row
FILL_W = 1568  # SBUF fill width; the output DMA replicates it
N_REP_DMA = COLS // FILL_W  # 4 replications per partition row

_CACHE = {}


FIRST_SLICE = 512  # first fill slice; its DMA starts while the rest fills
# (cost-model sweep: S0 480-512 is the flat optimum at 15262 ns/core; smaller
# starves the DMA pipe behind the second fill, larger delays the first DMA)


def _build_bass():
    """Per-core graph: out[p, :] = fb[p] for a [128, 6272] f32 block.

    Raw Bass (no Tile; Tile's kernel-tail Drain trips a codegen sync-wait
    limit here and its barriers cost more than this whole kernel).

    The SP sequencer issues the 512 B bias load; the DVE broadcast-fills a
    [128, 1568] SBUF chunk in two slices (512 + 1056 cols); SP issues the
    output DMAs as each slice lands: a head DMA for cols [0:512), a mid
    DMA for [512:1568), and one bulk DMA for [1568:6272) that reads the
    chunk three times per partition row (stride-0 middle dim). Bulk
    descriptors are 6272 B, above the 4 KiB small-descriptor HBM penalty
    threshold. TimelineSim: 15262 ns/core; the floor is ~8.9 us DMA wire
    (3.2 MB at 360 B/ns) + 2.26 us input-DMA fixed latency (625 HWDGE +
    650 DGE + 900 sem-prop) + 1.3 us program start + one DMA issue.
    Verified race-free and bit-exact in CoreSim.
    """
    f32 = mybir.dt.float32
    S0, W = FIRST_SLICE, FILL_W
    nc = bass.Bass("TRN2", debug=False)
    fb_in = nc.dram_tensor("fb", [P, 1], f32, kind="ExternalInput")
    out = nc.dram_tensor("out", [P, COLS], f32, kind="ExternalOutput")

    with (
        nc.sbuf_tensor("fb_sb", [P, 1], f32) as fb_sb,
        nc.sbuf_tensor("chunk", [P, W], f32) as chunk,
        nc.semaphore("dma_sem") as dma_sem,
        nc.semaphore("fill_sem") as fill_sem,
        nc.Block() as block,
    ):

        @block.sync
        def _(sync):
            sync.dma_start(out=fb_sb[:, :], in_=fb_in[:, :]).then_inc(dma_sem, 16)
            sync.wait_ge(fill_sem, 1)
            sync.dma_start(out=out.ap()[:, 0:S0], in_=chunk[:, 0:S0]).then_inc(
                dma_sem, 16
            )
            sync.wait_ge(fill_sem, 2)
            sync.dma_start(out=out.ap()[:, S0:W], in_=chunk[:, S0:W]).then_inc(
                dma_sem, 16
            )
            out_v = out.ap()[:, W:COLS].rearrange("p (r w) -> p r w", w=W)
            sync.dma_start(
                out=out_v,
                in_=chunk[:, None, :].broadcast_to([P, N_REP_DMA - 1, W]),
            ).then_inc(dma_sem, 16)
            sync.wait_ge(dma_sem, 64)

        @block.vector
        def _(vector):
            vector.wait_ge(dma_sem, 16)
            vector.tensor_copy(
                out=chunk[:, 0:S0], in_=fb_sb[:, 0:1].broadcast_to([P, S0])
            ).then_inc(fill_sem, 1)
            vector.tensor_copy(
                out=chunk[:, S0:W], in_=fb_sb[:, 0:1].broadcast_to([P, W - S0])
            ).then_inc(fill_sem, 1)

    return nc


# Stashed BassKernelResults from the most recent run (exec_time_ns etc.);
# used by the dev harness, not by grading.
LAST_RUN = None


def kernel(**inputs) -> np.ndarray:
    global LAST_RUN
    f_b = np.ascontiguousarray(np.asarray(inputs["f_b"]), dtype=np.float32)
    assert f_b.shape == (OUTC,), f_b.shape

    # Shard channels across cores; lay each core's 32 channels out over all
    # 128 partitions (4 partitions per channel).
    in_maps = []
    for ci in range(N_CORES):
        shard = f_b[ci * CPC : (ci + 1) * CPC]
        in_maps.append({"fb": np.repeat(shard, REP).reshape(P, 1).copy()})

    if "nc" not in _CACHE:
        _CACHE["nc"] = _build_bass()
    res = run_bass_kernel_spmd(_CACHE["nc"], in_maps, core_ids=list(range(N_CORES)))
    LAST_RUN = res

    # Unshard: per-core [128, 6272] -> [32, 25088]; concat channel blocks.
    parts = [np.asarray(r["out"]).reshape(CPC, SPATIAL) for r in res.results]
    return np.concatenate(parts, axis=0).reshape(OUT_SHAPE)
